# revision 7
# baseline (speedup 1.0000x reference)
"""NNConv (edge-conditioned graph conv) Trainium2 kernel, 8-core SPMD.

Strategy: edges are dst-sorted on host and bucketed into 8 contiguous
node ranges (1250 nodes/core), so each core owns a disjoint slice of the
output and no cross-core reduction is needed.  Per core:
  PE      : z = [ea;1]^T @ [W_edge;b]  (per-edge 32x32 weight logits)
  ACT     : relu + f32->bf16 evacuation of PSUM
  DVE     : y = relu(z) * x_src broadcast  (per-edge message terms, bf16 2x)
  DVE/GPS : sum_i y[:, (o,i)] -> msg[t, o]  (+ count col = 1), split for balance
  DVE     : per-partition prefix scan (bf16 msg in, f32 prefix out)
  PE      : strict-lower-triangular matmul for the cross-partition carry
  DMA     : prefix table P -> HBM; indirect row gathers at segment bounds
  PE/DVE  : aggr = (P[e_n]-P[e_{n-1}]) / max(cnt,1) + x@root + bias

Host-side cost is kept low: one quick argsort, np.take gathers in bf16,
and all per-core tensors are built directly into the concatenated
[8*dim0, ...] buffers the PJRT shard_map path consumes, with a cached
jitted executable (no per-call retrace).
"""

import os
import sys

sys.path.insert(0, "/opt/trn_rl_repo")

import numpy as np
import ml_dtypes

import concourse.bass as bass
import concourse.bacc as bacc
import concourse.mybir as mybir
import concourse.tile as tile

F32 = mybir.dt.float32
BF16 = mybir.dt.bfloat16
FP8 = mybir.dt.float8e4
I16 = mybir.dt.int16
I32 = mybir.dt.int32
BF = ml_dtypes.bfloat16
F8 = ml_dtypes.float8_e4m3

STAGE = int(os.environ.get("KSTAGE", "9"))
REPS = int(os.environ.get("KREPS", "1"))
SKIP = set(os.environ.get("KSKIP", "").split(","))
KLVL = int(os.environ.get("KLVL", "5"))
KMUL = os.environ.get("KMUL", "gps")     # multiply placement: dve|gps|split
KMULQ = int(os.environ.get("KMULQ", "2"))  # of every 5 tiles, this many mult on gpsimd
KMSG = os.environ.get("KMSG", "bf16")    # msg accumulator dtype: bf16|f32
KEAT = os.environ.get("KEAT", "dev")     # ea transpose: dev (PE) | host
KHALVE = os.environ.get("KHALVE", "1")   # 2x-mode halving add before the reduce
KXBC = os.environ.get("KXBC", "1")       # x table: 1 = shard H2D + AllGather on device
KHQ = int(os.environ.get("KHQ", "0"))    # of every 9 tiles, halving-adds on gpsimd
KEVS = os.environ.get("KEVS", "0")       # 1 = alternate transpose evac between DVE and ACT
KEA8 = os.environ.get("KEA8", "0")       # ship edge attrs as fp8e4m3, cast on device
KOUT16 = os.environ.get("KOUT16", "1")   # bf16 output + host upcast (halves D2H)

# problem constants (hardcoded per the harness contract)
N_NODES = 10000
IN_C = 32
OUT_C = 32
EDGE_F = 16
N_EDGES = 320000
CORES = 8
NPC = N_NODES // CORES          # 1250 nodes per core
NPT = 10                        # node tiles per core (128 each, padded 1280)
NPAD = 128 * NPT
JT = 328                        # free-dim edge slots per partition
EPC = 128 * JT                  # 41984 padded edge slots per core
IDXW = EPC // 16                # 2624
GCH = 41                        # x-gather chunks (1024 idxs each: SWDGE ring cap)
JCH = JT // GCH                 # 8 tiles per chunk
MC = 33                         # msg cols: 32 outputs + count
NTT = 79                        # node tiles for the padded x table
NRW = 128 * NTT                 # 10112 padded x rows

# slot s (SBUF column) -> sorted-edge position t; partition-minor so the
# per-partition prefix scan walks t in order within each partition
_S_ALL = np.arange(EPC)
T_OF_S = (_S_ALL % 128) * JT + _S_ALL // 128

_CACHE = {}


def _build_nc():
    nc = bacc.Bacc("TRN2", target_bir_lowering=False, debug=False,
                   num_devices=CORES)

    EADT = FP8 if KEA8 == "1" else BF16
    if KEAT == "dev":
        ea_d = nc.declare_dram_parameter("ea", [EPC, EDGE_F], EADT, isOutput=False)
    else:
        ea_d = nc.declare_dram_parameter("ea", [EDGE_F, EPC], EADT, isOutput=False)
    gidx_d = nc.declare_dram_parameter("gidx", [16, IDXW], I16, isOutput=False)
    if KXBC == "1":
        xg_d = nc.declare_dram_parameter("xg", [NRW // CORES, IN_C], BF16, isOutput=False)
    else:
        xg_d = nc.declare_dram_parameter("xg", [NRW, IN_C], BF16, isOutput=False)
    w_d = nc.declare_dram_parameter("wmat", [EDGE_F + 1, 1024], BF16, isOutput=False)
    bidx_d = nc.declare_dram_parameter("bidx", [128, NPT], I32, isOutput=False)
    pidx_d = nc.declare_dram_parameter("pidx", [128, NPT], I32, isOutput=False)
    xtb_d = nc.declare_dram_parameter("xtb", [IN_C + 1, NPAD], BF16, isOutput=False)
    rootb_d = nc.declare_dram_parameter("rootb", [IN_C + 1, OUT_C], BF16, isOutput=False)
    ODT = BF16 if KOUT16 == "1" else F32
    out_d = nc.declare_dram_parameter("out", [NPAD, OUT_C], ODT, isOutput=True)

    p_hbm = nc.dram_tensor("pfx", [EPC + 128, MC], F32)
    xg2_hbm = nc.dram_tensor("xg2", [NRW, 128], BF16)

    with tile.TileContext(nc) as tc:
        with (
            tc.tile_pool(name="const", bufs=1) as cpool,
            tc.tile_pool(name="big", bufs=1) as bigpool,
            tc.tile_pool(name="xsp", bufs=2) as xspool,
            tc.tile_pool(name="zp", bufs=2, space="PSUM") as zpsum,
            tc.tile_pool(name="work", bufs=3) as wpool,
            tc.tile_pool(name="small", bufs=1) as spool,
            tc.tile_pool(name="sps", bufs=1, space="PSUM") as spsum,
        ):
            # ---- resident tiles ----
            w_t = cpool.tile([EDGE_F + 1, 1024], BF16)
            nc.sync.dma_start(w_t[:], w_d[:])
            # ea features on partitions 0..15, ones row (for the bias) on 16.
            # memset the whole tile to 1.0 (base partition must be 0); the
            # transpose evacuations below overwrite rows 0..15.
            ea_t = bigpool.tile([EDGE_F + 1, EPC], BF16)
            nc.gpsimd.memset(ea_t[:], 1.0)
            if KEAT == "dev":
                # row-major edge attrs in, feature-major via PE transpose
                ea_rm = bigpool.tile([128, JT * EDGE_F], BF16)
                if KEA8 == "1":
                    ea_r8 = bigpool.tile([128, JT * EDGE_F], FP8)
                    nc.sync.dma_start(
                        ea_r8[:], ea_d[:].rearrange("(p j) f -> p (j f)", p=128)
                    )
                    nc.gpsimd.tensor_copy(ea_rm[:], ea_r8[:])
                else:
                    nc.sync.dma_start(
                        ea_rm[:], ea_d[:].rearrange("(p j) f -> p (j f)", p=128)
                    )
                ident = cpool.tile([128, 128], BF16)
                nc.gpsimd.memset(ident[:], 1.0)
                nc.gpsimd.affine_select(
                    ident[:], ident[:], [[-1, 128]],
                    mybir.AluOpType.is_equal, 0.0,
                    base=0, channel_multiplier=1,
                )
                with tc.tile_pool(name="tp", bufs=2, space="PSUM") as tpsum:
                    for g in range(JT // 4):
                        tp = tpsum.tile([EDGE_F, 512], BF16)
                        for jj in range(4):
                            j4 = g * 4 + jj
                            nc.tensor.transpose(
                                tp[:, jj * 128:(jj + 1) * 128],
                                ea_rm[:, j4 * EDGE_F:(j4 + 1) * EDGE_F],
                                ident[:],
                            )
                        ev_eng = nc.scalar if (KEVS == "1" and g % 2 == 0) else nc.vector
                        if ev_eng is nc.scalar:
                            nc.scalar.copy(
                                ea_t[0:EDGE_F, g * 512:(g + 1) * 512], tp[:]
                            )
                        else:
                            nc.vector.tensor_copy(
                                ea_t[0:EDGE_F, g * 512:(g + 1) * 512], tp[:]
                            )
            else:
                nc.sync.dma_start(ea_t[0:EDGE_F, :], ea_d[:])
            gidx_t = cpool.tile([128, IDXW], I16)
            for k16 in range(8):
                nc.sync.dma_start(gidx_t[16 * k16:16 * (k16 + 1), :], gidx_d[:])

            msg_t = bigpool.tile([128, JT, MC], BF16 if KMSG == "bf16" else F32)
            # count column = 1.0 for every slot
            nc.gpsimd.memset(
                msg_t[:].rearrange("p j c -> p (j c)")[:, OUT_C::MC], 1.0
            )
            pfx_t = bigpool.tile([128, JT, MC], F32)

            # zero row(s) of the prefix table (used by empty-segment bounds)
            zrow = spool.tile([128, MC], F32)
            nc.gpsimd.memset(zrow[:], 0.0)
            nc.sync.dma_start(
                p_hbm[EPC:EPC + 128, :], zrow[:]
            )

            # expand x to 256B rows in HBM (4 replicas) for the SWDGE gather
            if KXBC == "1":
                # each core uploaded 1/8 of the node table; gather the rest
                # over D2D instead of paying 8x H2D
                xgall_hbm = nc.dram_tensor("xgall", [NRW, IN_C], BF16)
                # collectives cannot read IO tensors; stage via internal DRAM
                xgin_hbm = nc.dram_tensor("xgin", [NRW // CORES, IN_C], BF16)
                nc.sync.dma_start(xgin_hbm[:], xg_d[:])
                nc.gpsimd.collective_compute(
                    "AllGather",
                    mybir.AluOpType.bypass,
                    replica_groups=[list(range(CORES))],
                    ins=[xgin_hbm[:].opt()],
                    outs=[xgall_hbm[:].opt()],
                )
                xg_src = xgall_hbm
            else:
                xg_src = xg_d
            xrep_t = cpool.tile([128, NTT, IN_C], BF16)
            nc.sync.dma_start(
                xrep_t[:], xg_src[:].rearrange("(t p) f -> p t f", p=128)
            )
            for r in range(4):
                nc.sync.dma_start(
                    xg2_hbm[:, r * IN_C:(r + 1) * IN_C]
                    .rearrange("(t p) f -> p t f", p=128),
                    xrep_t[:],
                )

            for _rep in range(REPS):
                # ---- main edge loop ----
                for c in range(GCH):
                    xs_t = xspool.tile([128, JCH, 128], BF16)
                    if "gather" not in SKIP:
                        nc.gpsimd.dma_gather(
                            xs_t[:], xg2_hbm[:],
                            gidx_t[:, c * (IDXW // GCH):(c + 1) * (IDXW // GCH)],
                            EPC // GCH, EPC // GCH, 128,
                        )
                    for jj in range(JCH):
                        j = c * JCH + jj
                        if KLVL < 2:
                            continue
                        z_ps = zpsum.tile([128, 1024], F32)
                        for h in range(2):
                            nc.tensor.matmul(
                                z_ps[:, h * 512:(h + 1) * 512],
                                ea_t[:, j * 128:(j + 1) * 128],
                                w_t[:, h * 512:(h + 1) * 512],
                                start=True, stop=True,
                            )
                        if KLVL < 3:
                            continue
                        zr_t = wpool.tile([128, 1024], BF16, tag="zr")
                        nc.scalar.activation(
                            zr_t[:], z_ps[:], mybir.ActivationFunctionType.Relu
                        )
                        if KLVL < 4:
                            continue
                        if KMUL == "gps" or (KMUL == "split" and j % 5 < KMULQ):
                            mul_eng = nc.gpsimd
                        else:
                            mul_eng = nc.vector
                        y_t = wpool.tile([128, 1024], BF16, tag="y")
                        mul_eng.tensor_tensor(
                            y_t[:].rearrange("p (o i) -> p o i", i=IN_C),
                            zr_t[:].rearrange("p (o i) -> p o i", i=IN_C),
                            xs_t[:, jj, 0:IN_C].unsqueeze(1).broadcast_to(
                                [128, OUT_C, IN_C]
                            ),
                            mybir.AluOpType.mult,
                        )
                        if KLVL < 5:
                            continue
                        yv = y_t[:].rearrange("p (o i) -> p o i", i=IN_C)
                        if KHALVE == "1":
                            # fold i-halves first: 2-byte packed -> DVE 2x mode
                            h_eng = nc.gpsimd if (j % 9 < KHQ) else nc.vector
                            h_t = wpool.tile([128, OUT_C, IN_C // 2], BF16, tag="h")
                            h_eng.tensor_tensor(
                                h_t[:], yv[:, :, 0:IN_C // 2],
                                yv[:, :, IN_C // 2:IN_C],
                                mybir.AluOpType.add,
                            )
                            yv = h_t[:]
                        with nc.allow_low_precision(reason="32-wide bf16 msg"):
                            nc.vector.tensor_reduce(
                                msg_t[:, j, 0:OUT_C],
                                yv,
                                mybir.AxisListType.X,
                                mybir.AluOpType.add,
                            )

                # ---- segment sum via prefix scan ----
                if STAGE >= 2:
                    # tri[k, m] = 1 iff m > k (strict upper triangle), built on
                    # device: iota = m - k - 1 >= 0 keeps the memset ones
                    tri_t = cpool.tile([128, 128], F32)
                    nc.gpsimd.memset(tri_t[:], 1.0)
                    nc.gpsimd.affine_select(
                        tri_t[:], tri_t[:], [[1, 128]],
                        mybir.AluOpType.is_ge, 0.0,
                        base=-1, channel_multiplier=-1,
                    )
                    tot_t = spool.tile([128, MC], F32)
                    nc.vector.tensor_reduce(
                        tot_t[:],
                        msg_t[:].rearrange("p j c -> p c j"),
                        mybir.AxisListType.X,
                        mybir.AluOpType.add,
                    )
                    carry_ps = spsum.tile([128, MC], F32)
                    nc.tensor.matmul(carry_ps[:], tri_t[:], tot_t[:], start=True, stop=True)
                    carry_t = spool.tile([128, MC], F32)
                    nc.vector.tensor_copy(carry_t[:], carry_ps[:])

                    zcol = spool.tile([128, 1], F32)
                    nc.gpsimd.memset(zcol[:], 0.0)
                    for cc in range(MC):
                        icol = msg_t[:].rearrange("p j c -> p c j")[:, cc, :]
                        ocol = pfx_t[:].rearrange("p j c -> p c j")[:, cc, :]
                        nc.vector.tensor_tensor_scan(
                            ocol, icol,
                            zcol[:].broadcast_to([128, JT]),
                            carry_t[:, cc:cc + 1],
                            mybir.AluOpType.add,
                            mybir.AluOpType.add,
                        )

                    nc.sync.dma_start(
                        p_hbm[0:EPC, :].rearrange("(p j) c -> p j c", j=JT), pfx_t[:]
                    )

                # ---- boundary gathers + final update ----
                if STAGE >= 3:
                    bidx_t = spool.tile([128, NPT], I32)
                    nc.sync.dma_start(bidx_t[:], bidx_d[:])
                    pidx_t = spool.tile([128, NPT], I32)
                    nc.sync.dma_start(pidx_t[:], pidx_d[:])
                    pb_t = spool.tile([128, NPT, MC], F32)
                    pp_t = spool.tile([128, NPT, MC], F32)
                    for j2 in range(NPT):
                        nc.gpsimd.indirect_dma_start(
                            pb_t[:, j2, :], None, p_hbm[:],
                            bass.IndirectOffsetOnAxis(ap=bidx_t[:, j2:j2 + 1], axis=0),
                        )
                        nc.gpsimd.indirect_dma_start(
                            pp_t[:, j2, :], None, p_hbm[:],
                            bass.IndirectOffsetOnAxis(ap=pidx_t[:, j2:j2 + 1], axis=0),
                        )
                    seg_t = spool.tile([128, NPT, MC], F32)
                    nc.vector.tensor_tensor(
                        seg_t[:], pb_t[:], pp_t[:], mybir.AluOpType.subtract
                    )
                    cnt_t = spool.tile([128, NPT], F32)
                    nc.vector.tensor_scalar_max(
                        cnt_t[:], seg_t[:, :, OUT_C], 1.0
                    )
                    rcp_t = spool.tile([128, NPT], F32)
                    nc.vector.reciprocal(rcp_t[:], cnt_t[:])

                    xtb_t = spool.tile([IN_C + 1, NPAD], BF16)
                    nc.sync.dma_start(xtb_t[:], xtb_d[:])
                    rootb_t = spool.tile([IN_C + 1, OUT_C], BF16)
                    nc.sync.dma_start(rootb_t[:], rootb_d[:])
                    rx_ps = spsum.tile([128, NPT * OUT_C], F32)
                    for j2 in range(NPT):
                        nc.tensor.matmul(
                            rx_ps[:, j2 * OUT_C:(j2 + 1) * OUT_C],
                            xtb_t[:, j2 * 128:(j2 + 1) * 128],
                            rootb_t[:],
                            start=True, stop=True,
                        )
                    fin_t = spool.tile([128, NPT * OUT_C], ODT)
                    for j2 in range(NPT):
                        nc.vector.scalar_tensor_tensor(
                            fin_t[:, j2 * OUT_C:(j2 + 1) * OUT_C],
                            seg_t[:, j2, 0:OUT_C],
                            rcp_t[:, j2:j2 + 1],
                            rx_ps[:, j2 * OUT_C:(j2 + 1) * OUT_C],
                            mybir.AluOpType.mult,
                            mybir.AluOpType.add,
                        )
                    nc.sync.dma_start(
                        out_d[:].rearrange("(j p) o -> p j o", p=128),
                        fin_t[:].rearrange("p (j o) -> p j o", o=OUT_C),
                    )

                if KLVL >= 5:
                    if STAGE < 3:
                        nc.gpsimd.dma_start(
                            out_d[:].rearrange("(j p) o -> p j o", p=128),
                            msg_t[:, 0:NPT, 0:OUT_C],
                        )
                    elif STAGE < 4:
                        nc.sync.dma_start(
                            out_d[:].rearrange("(j p) o -> p j o", p=128),
                            pb_t[:, :, 0:OUT_C],
                        )

    nc.compile()
    return nc


def _build():
    """Compile the Bass module once and cache a jitted SPMD executable.

    Mirrors concourse.bass2jax.run_bass_via_pjrt, but the jax.jit wrapper
    is constructed a single time so repeat kernel() calls skip retracing.
    """
    if "runner" in _CACHE:
        return _CACHE["runner"]

    import jax
    from jax.sharding import Mesh, PartitionSpec
    from jax.experimental.shard_map import shard_map
    from concourse import bass2jax
    from concourse._compat import axon_active

    nc = _build_nc()

    if not axon_active():
        # native path: run_bass_kernel_spmd drives NRT directly
        from concourse.bass_utils import run_bass_kernel_spmd

        def run_native(in_map):
            per_core = [
                {n: np.asarray(v).reshape(
                    CORES, np.asarray(v).shape[0] // CORES,
                    *np.asarray(v).shape[1:])[k]
                 for n, v in in_map.items()}
                for k in range(CORES)
            ]
            res = run_bass_kernel_spmd(nc, per_core, list(range(CORES)))
            out = np.concatenate(
                [res.results[k]["out"] for k in range(CORES)], axis=0)
            return {"out": out}

        _CACHE["runner"] = (nc, run_native, None)
        return _CACHE["runner"]
    bass2jax.install_neuronx_cc_hook()
    assert nc.dbg_addr is None

    partition_name = nc.partition_id_tensor.name if nc.partition_id_tensor else None
    in_names, out_names, out_avals = [], [], []
    for alloc in nc.m.functions[0].allocations:
        if not isinstance(alloc, mybir.MemoryLocationSet):
            continue
        name = alloc.memorylocations[0].name
        if alloc.kind == "ExternalInput":
            if name != partition_name:
                in_names.append(name)
        elif alloc.kind == "ExternalOutput":
            shape = tuple(alloc.tensor_shape)
            dtype = mybir.dt.np(alloc.dtype)
            out_names.append(name)
            out_avals.append(jax.core.ShapedArray(shape, dtype))
    n_params = len(in_names)
    n_outs = len(out_avals)
    all_names = list(in_names) + list(out_names)
    if partition_name is not None:
        all_names.append(partition_name)
    donate = tuple(range(n_params, n_params + n_outs))

    def _body(*args):
        operands = list(args)
        if partition_name is not None:
            operands.append(bass2jax.partition_id_tensor())
        outs = bass2jax._bass_exec_p.bind(
            *operands,
            out_avals=tuple(out_avals),
            in_names=tuple(all_names),
            out_names=tuple(out_names),
            lowering_input_output_aliases=(),
            sim_require_finite=True,
            sim_require_nnan=True,
            nc=nc,
        )
        return tuple(outs)

    devices = jax.devices()[:CORES]
    assert len(devices) == CORES, f"need {CORES} devices, have {len(jax.devices())}"
    mesh = Mesh(np.asarray(devices), ("core",))
    in_specs = (PartitionSpec("core"),) * (n_params + n_outs)
    out_specs = (PartitionSpec("core"),) * n_outs
    sharded = jax.jit(
        shard_map(_body, mesh=mesh, in_specs=in_specs, out_specs=out_specs,
                  check_rep=False),
        donate_argnums=donate, keep_unused=True,
    )
    out_shapes = [(CORES * a.shape[0], *a.shape[1:]) for a in out_avals]
    out_dtypes = [a.dtype for a in out_avals]

    from jax.sharding import NamedSharding
    sh = NamedSharding(mesh, PartitionSpec("core"))

    def put(arr):
        # async H2D with the sharding the jitted call expects (zero-copy)
        return jax.device_put(arr, sh)

    def run(in_map):
        # donated result buffers; the kernel writes every element of out.
        # Reuse the previous call's device-resident outputs when possible
        # to skip their H2D entirely.
        bufs = _CACHE.pop("prev_outs", None)
        if bufs is None:
            # committed like prev_outs so the jit compiles a single variant
            bufs = [put(np.empty(s, d))
                    for s, d in zip(out_shapes, out_dtypes)]
        outs = sharded(*[in_map[n] for n in in_names], *bufs)
        res = {n: np.asarray(outs[i]) for i, n in enumerate(out_names)}
        _CACHE["prev_outs"] = list(outs)
        return res

    _CACHE["runner"] = (nc, run, put)
    return _CACHE["runner"]


def _prep_inputs(x, edge_index, edge_attr, W_edge, b_edge, root, bias,
                 put=None):
    """Host-side sharding: dst-sort, bucket by node range, build the
    concatenated [8*dim0, ...] physical buffers directly.

    When `put` is given, each finished buffer is handed to it immediately
    (async H2D) so the large ea transfer overlaps the rest of the prep.
    """
    if put is None:
        put = lambda a: a
    src = np.asarray(edge_index[0]).astype(np.int32, copy=False)
    dst = np.asarray(edge_index[1]).astype(np.int32, copy=False)
    ea = np.asarray(edge_attr, dtype=np.float32)
    x = np.asarray(x, dtype=np.float32)

    # cast ea concurrently with the sort (both release the GIL)
    import threading
    ea_box = {}

    def _cast_ea():
        ea_box["v"] = ea.astype(F8 if KEA8 == "1" else BF)

    th = threading.Thread(target=_cast_ea)
    th.start()

    order = np.argsort(dst).astype(np.int32)
    # per-node global edge boundaries in the sorted order (bincount is
    # cheaper than searchsorted on the sorted copy)
    csum = np.zeros(N_NODES + 1, np.int64)
    np.cumsum(np.bincount(dst, minlength=N_NODES), out=csum[1:])
    bounds = csum[::NPC]  # [9]

    # padded per-core sorted edge ids [8, EPC]
    ids_pad = np.empty((CORES, EPC), np.int32)
    for k in range(CORES):
        lo, hi = int(bounds[k]), int(bounds[k + 1])
        m = hi - lo
        assert m <= EPC, f"core {k} edge count {m} > {EPC}"
        ids_pad[k, :m] = order[lo:hi]
        ids_pad[k, m:] = order[lo] if m else 0

    # edge attrs first: biggest transfer, start it before the rest of prep
    th.join()
    ea16 = ea_box["v"]
    if KEAT == "dev":
        # HBM row t (= p*JT + j) is sorted position t directly
        eaC = put(np.take(ea16, ids_pad.reshape(-1), axis=0))  # [8*EPC, 16]
    else:
        ea_g = np.take(ea16, ids_pad[:, T_OF_S].reshape(-1), axis=0)
        eaT = np.empty((CORES * EDGE_F, EPC), F8 if KEA8 == "1" else BF)
        for k in range(CORES):
            eaT[k * EDGE_F:(k + 1) * EDGE_F] = ea_g[k * EPC:(k + 1) * EPC].T
        eaC = put(eaT)

    col_all = ids_pad[:, T_OF_S].reshape(-1)  # [8*EPC] edge id per SBUF column

    # x-gather indices: wrap in 16 partitions (device replicates x8)
    srcg = src[col_all].astype(np.int16).reshape(CORES, EPC // 16, 16)
    gidxC = put(np.ascontiguousarray(
        srcg.transpose(0, 2, 1)).reshape(CORES * 16, IDXW))

    # node features padded to full 128-row tiles; with the device-side
    # AllGather each core uploads only its 1/8 shard (the concat buffer IS
    # the full table), otherwise every core gets a full copy
    xg16 = x.astype(BF)
    xgp = np.zeros((NRW, IN_C), BF)
    xgp[:N_NODES] = xg16
    xgC = put(xgp if KXBC == "1" else np.tile(xgp, (CORES, 1)))
    W = np.asarray(W_edge, dtype=np.float32).reshape(EDGE_F, IN_C, OUT_C)
    b = np.asarray(b_edge, dtype=np.float32).reshape(IN_C, OUT_C)
    wmat = np.empty((EDGE_F + 1, 1024), dtype=np.float32)
    wmat[:EDGE_F] = W.transpose(0, 2, 1).reshape(EDGE_F, 1024)
    wmat[EDGE_F] = b.T.reshape(1024)
    wmatC = put(np.tile(wmat.astype(BF), (CORES, 1)))
    rootb = np.concatenate(
        [np.asarray(root, np.float32),
         np.asarray(bias, np.float32)[None, :]], axis=0
    ).astype(BF)
    rootbC = put(np.tile(rootb, (CORES, 1)))

    # segment boundary rows (last edge of node n / of node n-1), local coords
    lo_node = np.repeat(bounds[:CORES].astype(np.int64), NPC)
    bv = csum[1:] - 1 - lo_node     # [10000]
    pv = csum[:-1] - 1 - lo_node
    bfull = np.full((CORES, NPAD), EPC, np.int32)
    pfull = np.full((CORES, NPAD), EPC, np.int32)
    bfull[:, :NPC] = np.where(bv >= 0, bv, EPC).reshape(CORES, NPC)
    pfull[:, :NPC] = np.where(pv >= 0, pv, EPC).reshape(CORES, NPC)
    bidxC = put(np.ascontiguousarray(
        bfull.reshape(CORES, NPT, 128).transpose(0, 2, 1)).reshape(CORES * 128, NPT))
    pidxC = put(np.ascontiguousarray(
        pfull.reshape(CORES, NPT, 128).transpose(0, 2, 1)).reshape(CORES * 128, NPT))

    # per-core node features (feature-major, bf16) + ones row for bias
    xT16 = np.ones((IN_C + 1, N_NODES), BF)
    xT16[:IN_C] = xg16.T
    xtbC = np.zeros((CORES * (IN_C + 1), NPAD), BF)
    for k in range(CORES):
        xtbC[k * (IN_C + 1):(k + 1) * (IN_C + 1), :NPC] = \
            xT16[:, k * NPC:(k + 1) * NPC]
    xtbC = put(xtbC)

    return {
        "ea": eaC, "gidx": gidxC, "xg": xgC, "wmat": wmatC,
        "bidx": bidxC, "pidx": pidxC, "xtb": xtbC, "rootb": rootbC,
    }


def kernel(**inputs) -> np.ndarray:
    nc, run, put = _build()
    in_map = _prep_inputs(**inputs, put=put)
    res = run(in_map)
    out = res["out"].reshape(CORES, NPAD, OUT_C)[:, :NPC]
    return np.ascontiguousarray(
        out.reshape(N_NODES, OUT_C)).astype(np.float32)
